# revision 1
# baseline (speedup 1.0000x reference)
"""Trainium2 Bass kernel for CoocOpModel.

out[b,s,z] = sum_{i,j} func[b,s,i] * cooc[i,j,z] * arg[b,s,j]
  with func = func_and_arg[..., :128], arg = func_and_arg[..., 128:]

Shapes (hardcoded): func_and_arg [4,1024,256] f32, cooccurrences [128,128,128] f32,
out [4,1024,128] f32.  D = 128, tokens T = 4096.

Strategy: data-parallel over tokens across 8 cores (512 tokens/core);
cooccurrence tensor replicated per core (fp16).

Per-core math, with t = local token index (512), i/j/z in [0,128):
  out_T[z, t] = sum_i  C_i^T @ G_i        (accumulated in one PSUM bank)
  C_i[j, z]   = cooc[i, j, z]             (stationary operand, fp16)
  G_i[j, t]   = arg_T[j, t] * func_T[i, t]  (moving operand, fp16)

The kernel is DMA-bus-bound (~340-420 GB/s/core, saturated by a single
queue; extra queues or engines don't add bandwidth, and heavier
multi-engine concurrency invites power throttling).  So the design
minimizes everything except the unavoidable ~20.75 MB of transfers and
keeps the bus busy 100% of the time:
  - ALL DMA dispatches are issued upfront (the async HW queues drain in
    the background), byte-balanced across the two HWDGE queues, with
    every f_exp tile having its own buffer so no dispatch ever waits on
    a tile-reuse semaphore.
  - i-groups are sized (4,4,8,...,8,4,4): small head groups so the first
    TT/matmul starts ~2 us earlier, small tail groups so the post-last-
    transfer drain is shorter.
  - one DVE tensor-tensor per group builds G (fp16 2x mode), re-reading
    arg_T per k through a free-dim step-0 AP; 128 accumulating matmuls.
"""

import sys

sys.path.insert(0, "/opt/trn_rl_repo")

import numpy as np
from contextlib import ExitStack

import concourse.bass as bass
import concourse.tile as tile
from concourse import bacc, mybir
from concourse.bass_utils import run_bass_kernel_spmd

F16 = mybir.dt.float16
F32 = mybir.dt.float32
NP_F16 = np.float16

N_CORES = 8
D = 128
T_TOTAL = 4096
T_CORE = T_TOTAL // N_CORES  # 512

SIZES = (4, 4) + (8,) * 14 + (4, 4)
assert sum(SIZES) == D

_NC_CACHE = None


def _build():
    nc = bacc.Bacc("TRN2", target_bir_lowering=False, debug=False, num_devices=N_CORES)

    f_t = nc.dram_tensor("f_t", [D, T_CORE], F16, kind="ExternalInput").ap()
    a_t = nc.dram_tensor("a_t", [D, T_CORE], F16, kind="ExternalInput").ap()
    # c2[j, i*128 + z] = cooc[i, j, z]
    c2 = nc.dram_tensor("c2", [D, D * D], F16, kind="ExternalInput").ap()
    out_t = nc.dram_tensor("out_t", [D, T_CORE], F32, kind="ExternalOutput").ap()

    n_grp = len(SIZES)
    starts = [sum(SIZES[:g]) for g in range(n_grp)]

    with tile.TileContext(nc) as tc:
        with ExitStack() as ctx:
            const_pool = ctx.enter_context(tc.tile_pool(name="const", bufs=1))
            fexp_pool = ctx.enter_context(tc.tile_pool(name="fexp", bufs=1))
            g_pool = ctx.enter_context(tc.tile_pool(name="g", bufs=3))
            out_pool = ctx.enter_context(tc.tile_pool(name="out", bufs=1))
            psum_pool = ctx.enter_context(
                tc.tile_pool(name="psum", bufs=1, space="PSUM")
            )

            # arg_T in SBUF; the TT re-reads it per k via a free-step-0 AP.
            a_sb = const_pool.tile([D, T_CORE], F16, tag="a")
            nc.sync.dma_start(a_sb[:], a_t[:, :])
            a_ap = a_sb[:]

            c_tiles = [
                const_pool.tile([D, SIZES[g] * D], F16, tag=f"c{g}", name=f"c_sb{g}")
                for g in range(n_grp)
            ]
            # every f_exp group gets its own buffer (tag-unique in a bufs=1
            # pool) so no DMA dispatch ever waits on a tile-reuse semaphore
            fexp_tiles = [
                fexp_pool.tile(
                    [D, SIZES[g] * T_CORE], F16, tag=f"f{g}", name=f"fexp{g}"
                )
                for g in range(n_grp)
            ]

            # ---- all DMA dispatches upfront, byte-balanced over queues ---
            q_bytes = [0, 0]
            dma_q = [nc.sync, nc.scalar]

            def issue(dst, src, nbytes, q=None):
                if q is None:
                    q = 0 if q_bytes[0] <= q_bytes[1] else 1
                dma_q[q].dma_start(dst, src)
                q_bytes[q] += nbytes

            cooc_issued = 0

            def issue_cooc(n):
                nonlocal cooc_issued
                for _ in range(n):
                    if cooc_issued >= n_grp:
                        return
                    g = cooc_issued
                    cooc_issued += 1
                    i0, sz = starts[g], SIZES[g]
                    issue(
                        c_tiles[g][:],
                        c2[:, i0 * D : (i0 + sz) * D],
                        sz * D * D * 2,
                    )

            for g in range(n_grp):
                i0, sz = starts[g], SIZES[g]
                fb = D * sz * T_CORE * 2
                if g == 0:
                    # split the head transfer across both queues so the
                    # pipeline's first TT starts as early as possible
                    half = sz // 2
                    f_src_a = bass.AP(
                        f_t.tensor, 0, [[0, D], [T_CORE, half], [1, T_CORE]]
                    )
                    f_src_b = bass.AP(
                        f_t.tensor,
                        half * T_CORE,
                        [[0, D], [T_CORE, half], [1, T_CORE]],
                    )
                    issue(fexp_tiles[0][:, : half * T_CORE], f_src_a, fb // 2, q=1)
                    issue(fexp_tiles[0][:, half * T_CORE :], f_src_b, fb // 2, q=0)
                else:
                    f_src = bass.AP(
                        f_t.tensor,
                        i0 * T_CORE,
                        [[0, D], [T_CORE, sz], [1, T_CORE]],
                    )
                    issue(fexp_tiles[g][:], f_src, fb)
                issue_cooc(1)

            ps = psum_pool.tile([D, T_CORE], F32)
            for g in range(n_grp):
                i0, sz = starts[g], SIZES[g]
                f_exp = fexp_tiles[g]

                a_view = bass.AP(
                    a_ap.tensor, a_ap.offset, [a_ap.ap[0], [0, sz], [1, T_CORE]]
                )
                gt = g_pool.tile([D, 8 * T_CORE], F16, tag="g")
                nc.vector.tensor_mul(gt[:, : sz * T_CORE], a_view, f_exp[:])

                for k in range(sz):
                    i = i0 + k
                    nc.tensor.matmul(
                        ps[:],
                        c_tiles[g][:, k * D : (k + 1) * D],
                        gt[:, k * T_CORE : (k + 1) * T_CORE],
                        start=(i == 0),
                        stop=(i == D - 1),
                    )

            o_sb = out_pool.tile([D, T_CORE], F32, tag="o")
            nc.scalar.copy(o_sb[:], ps[:])
            nc.sync.dma_start(out_t[:, :], o_sb[:])

    nc.compile()
    return nc


def _get_nc():
    global _NC_CACHE
    if _NC_CACHE is None:
        _NC_CACHE = _build()
    return _NC_CACHE


def _prep_in_maps(func_and_arg, cooccurrences):
    fa = np.asarray(func_and_arg, dtype=np.float32).reshape(T_TOTAL, 2 * D)
    c2 = (
        np.ascontiguousarray(
            np.asarray(cooccurrences, dtype=np.float32).transpose(1, 0, 2)
        )
        .reshape(D, D * D)
        .astype(NP_F16)
    )
    in_maps = []
    for c in range(N_CORES):
        s = fa[c * T_CORE : (c + 1) * T_CORE]  # [512, 256]
        f_tc = np.ascontiguousarray(s[:, :D].T).astype(NP_F16)  # [128 i, 512 t]
        a_tc = np.ascontiguousarray(s[:, D:].T).astype(NP_F16)  # [128 j, 512 t]
        in_maps.append({"f_t": f_tc, "a_t": a_tc, "c2": c2})
    return in_maps


def kernel(func_and_arg: np.ndarray, cooccurrences: np.ndarray) -> np.ndarray:
    assert func_and_arg.shape == (4, 1024, 2 * D)
    assert cooccurrences.shape == (D, D, D)

    in_maps = _prep_in_maps(func_and_arg, cooccurrences)
    nc = _get_nc()
    res = run_bass_kernel_spmd(nc, in_maps, core_ids=list(range(N_CORES)))

    # out_t per core: [z=128, t=512] -> [t, z]; concat over cores -> [4096, 128]
    outs = [res.results[c]["out_t"].T for c in range(N_CORES)]
    out = np.concatenate(outs, axis=0).reshape(4, 1024, D).astype(np.float32)
    return out



# revision 6
# speedup vs baseline: 1.4664x; 1.4664x over previous
"""Trainium2 Bass kernel for CoocOpModel.

out[b,s,z] = sum_{i,j} func[b,s,i] * cooc[i,j,z] * arg[b,s,j]
  with func = func_and_arg[..., :128], arg = func_and_arg[..., 128:]

Shapes (hardcoded): func_and_arg [4,1024,256] f32, cooccurrences [128,128,128] f32,
out [4,1024,128] f32.  D = 128, tokens T = 4096.

Strategy: data-parallel over tokens across 8 cores (512 tokens/core).

Per-core math (t = local token index in [0,512)):
  Lane packing: each SBUF partition (matmul contraction lane) is a pair
  lane = (i_sub, j_sub), i_sub in [0,8), j_sub in [0,16).
  Matmul m = (b, c), b in [0,16), c in [0,8) contracts 128 (i,j) pairs:
    i = 8*b + i_sub,  j = 16*c + j_sub
    out[z, t] += sum_lane  c3[lane, m, z] * G_m[lane, t]
    c3[lane, (m, z)] = cooc[8b+i_sub, 16c+j_sub, z]     (host-rearranged)
    G_m[lane, t]     = f[8b+i_sub, t] * a[16c+j_sub, t] (DVE tensor_tensor)
  via replicated operands in SBUF:
    f_rep[lane, (b, t)] = f[8b+i_sub, t]   (2 MB, 16x replication over j_sub)
    a_rep[lane, (c, t)] = a[16c+j_sub, t]  (1 MB,  8x replication over i_sub)
  so total DMA is ~7.25 MB/core instead of 20.4 MB/core for the naive
  1-i-per-matmul layout (which needs f broadcast to all 128 partitions).

Bottleneck model: DVE tensor_tensor builds G (65536 f16 elems/lane at 2x
mode ~= 34 us) and paces the PE (128 matmuls at ~216 ns warm ~= 28 us).
DMA ~7.25 MB ~= 18-20 us hides under the DVE span.  TT chunks are one
b-block (FD=4096) except the first b is split in half so the pipeline
fills early; all DMA dispatches are issued upfront on the two HWDGE
queues ordered so the first TT's inputs land first.
"""

import sys

sys.path.insert(0, "/opt/trn_rl_repo")

import numpy as np
from contextlib import ExitStack

import concourse.bass as bass
import concourse.tile as tile
from concourse import bacc, mybir
from concourse.bass_utils import run_bass_kernel_spmd

F16 = mybir.dt.float16
F32 = mybir.dt.float32
NP_F16 = np.float16

N_CORES = 8
D = 128
T_TOTAL = 4096
T_CORE = T_TOTAL // N_CORES  # 512

P_I = 8    # i_sub values per lane group
P_J = 16   # j_sub values
NB = 16    # b blocks: i = 8b + i_sub
NCC = 8    # c blocks: j = 16c + j_sub

_NC_CACHE = None


def _build():
    nc = bacc.Bacc("TRN2", target_bir_lowering=False, debug=False, num_devices=N_CORES)

    # host-replicated operands (see _prep_in_maps):
    #   f_in[lane, b*512+t] = f[8b+i_sub, t],  a_in[lane, c*512+t] = a[16c+j_sub, t]
    f_in = nc.dram_tensor("f_rep", [D, NB * T_CORE], F16, kind="ExternalInput").ap()
    a_in = nc.dram_tensor("a_rep", [D, NCC * T_CORE], F16, kind="ExternalInput").ap()
    # c3[lane, m*128 + z] = cooc[8b+i_sub, 16c+j_sub, z], lane=(i_sub,j_sub), m=(b,c)
    c3 = nc.dram_tensor("c3", [D, D * D], F16, kind="ExternalInput").ap()
    out_t = nc.dram_tensor("out_t", [D, T_CORE], F32, kind="ExternalOutput").ap()

    with tile.TileContext(nc) as tc:
        with ExitStack() as ctx:
            const_pool = ctx.enter_context(tc.tile_pool(name="const", bufs=1))
            g_pool = ctx.enter_context(tc.tile_pool(name="g", bufs=3))
            out_pool = ctx.enter_context(tc.tile_pool(name="out", bufs=1))
            psum_pool = ctx.enter_context(
                tc.tile_pool(name="psum", bufs=1, space="PSUM")
            )

            a_rep = const_pool.tile([D, NCC * T_CORE], F16, tag="arep")  # [lane,(c,t)]
            f_rep = const_pool.tile([D, NB * T_CORE], F16, tag="frep")   # [lane,(b,t)]
            c_sb = const_pool.tile([D, D * D], F16, tag="c3")            # [lane,(m,z)]

            # ---- all DMA dispatches upfront ------------------------------
            # q0 = sync, q1 = scalar (two HWDGE queues, FIFO each).
            # First TT chunk needs f_rep[b=0] + a_rep[c=0..3]; MMs of b=0
            # need c3[m=0..7].  Order queues so those land first.
            q0, q1 = nc.sync, nc.scalar

            # q1: f_rep b=0 (128 KB) first, then c3 in 2-b chunks (512 KB).
            q1.dma_start(f_rep[:, 0:T_CORE], f_in[:, 0:T_CORE])
            for k in range(8):  # c3 m-chunks of 16 (2 b's each)
                m0 = k * 16
                q1.dma_start(c_sb[:, m0 * D : (m0 + 16) * D], c3[:, m0 * D : (m0 + 16) * D])

            # q0: a_rep in two c-halves, then remaining f_rep b-blocks.
            q0.dma_start(a_rep[:, 0 : 4 * T_CORE], a_in[:, 0 : 4 * T_CORE])
            q0.dma_start(a_rep[:, 4 * T_CORE :], a_in[:, 4 * T_CORE :])
            for b0, b1 in ((1, 4), (4, 10), (10, 16)):
                q0.dma_start(
                    f_rep[:, b0 * T_CORE : b1 * T_CORE],
                    f_in[:, b0 * T_CORE : b1 * T_CORE],
                )

            # ---- compute: TT chunk -> matmuls, accumulate in one PSUM bank
            # chunks: (b, c0, c1)
            chunks = [(0, 0, 4), (0, 4, 8)] + [(b, 0, 8) for b in range(1, NB)]

            ps = psum_pool.tile([D, T_CORE], F32)
            f_ap = f_rep[:]
            for b, c0, c1 in chunks:
                ncol = (c1 - c0) * T_CORE
                gt = g_pool.tile([D, NCC * T_CORE], F16, tag="g")
                # G[lane, (c, t)] = a_rep[lane, (c, t)] * f_rep[lane, (b fixed, t)]
                f_view = bass.AP(
                    f_ap.tensor,
                    f_ap.offset + b * T_CORE,
                    [f_ap.ap[0], [0, c1 - c0], [1, T_CORE]],
                )
                nc.vector.tensor_mul(
                    gt[:, 0:ncol],
                    a_rep[:, c0 * T_CORE : c1 * T_CORE],
                    f_view,
                )
                for c in range(c0, c1):
                    m = b * NCC + c
                    nc.tensor.matmul(
                        ps[:],
                        c_sb[:, m * D : (m + 1) * D],
                        gt[:, (c - c0) * T_CORE : (c - c0 + 1) * T_CORE],
                        start=(m == 0),
                        stop=(m == D - 1),
                    )

            o_sb = out_pool.tile([D, T_CORE], F32, tag="o")
            nc.scalar.copy(o_sb[:], ps[:])
            nc.sync.dma_start(out_t[:, :], o_sb[:])

    nc.compile()
    return nc


def _get_nc():
    global _NC_CACHE
    if _NC_CACHE is None:
        _NC_CACHE = _build()
    return _NC_CACHE


def _prep_in_maps(func_and_arg, cooccurrences):
    fa = np.asarray(func_and_arg, dtype=np.float32).reshape(T_TOTAL, 2 * D)
    c = np.asarray(cooccurrences, dtype=np.float32)
    # c3[(i_sub, j_sub), (b, c, z)] = cooc[8b+i_sub, 16c+j_sub, z]
    c3 = np.ascontiguousarray(
        c.reshape(NB, P_I, NCC, P_J, D).transpose(1, 3, 0, 2, 4).reshape(D, D * D)
    ).astype(NP_F16)
    in_maps = []
    for core in range(N_CORES):
        s = fa[core * T_CORE : (core + 1) * T_CORE]  # [512, 256]
        f_tc = s[:, :D].T.astype(NP_F16)  # [128 i, 512 t]
        a_tc = s[:, D:].T.astype(NP_F16)  # [128 j, 512 t]
        # f_rep[(i_sub, j_sub), (b, t)] = f[8b+i_sub, t]
        f_rep = np.ascontiguousarray(
            np.broadcast_to(
                f_tc.reshape(NB, P_I, T_CORE).transpose(1, 0, 2)[:, None, :, :],
                (P_I, P_J, NB, T_CORE),
            )
        ).reshape(D, NB * T_CORE)
        # a_rep[(i_sub, j_sub), (c, t)] = a[16c+j_sub, t]
        a_rep = np.ascontiguousarray(
            np.broadcast_to(
                a_tc.reshape(NCC, P_J, T_CORE).transpose(1, 0, 2)[None, :, :, :],
                (P_I, P_J, NCC, T_CORE),
            )
        ).reshape(D, NCC * T_CORE)
        in_maps.append({"f_rep": f_rep, "a_rep": a_rep, "c3": c3})
    return in_maps


def kernel(func_and_arg: np.ndarray, cooccurrences: np.ndarray) -> np.ndarray:
    assert func_and_arg.shape == (4, 1024, 2 * D)
    assert cooccurrences.shape == (D, D, D)

    in_maps = _prep_in_maps(func_and_arg, cooccurrences)
    nc = _get_nc()
    res = run_bass_kernel_spmd(nc, in_maps, core_ids=list(range(N_CORES)))

    # out_t per core: [z=128, t=512] -> [t, z]; concat over cores -> [4096, 128]
    outs = [res.results[c]["out_t"].T for c in range(N_CORES)]
    out = np.concatenate(outs, axis=0).reshape(4, 1024, D).astype(np.float32)
    return out
